# revision 3
# baseline (speedup 1.0000x reference)
"""Trainium kernel for nn_CriterionMatching: CE + class-matching loss.

Strategy (per spec sharding_hint): pure data parallel — the batch dim
(B=16) is sharded across the 8 NeuronCores with jax shard_map; each core
computes its 2 samples' CE sum and matching terms entirely on-device;
the tiny per-sample scalars are gathered and combined on host.

Key algebraic reductions (vs the reference):
 - mean(s1.T @ s1a) over the 400x400 pairs == dot(mean_col(s1), mean_col(s1a)),
   so no pairwise matrix is ever formed.
 - top-400-by-key selection == thresholding at the 400th order statistic.
   The threshold is found with a 40-step vectorized bisection on-device
   (keys are continuous, so count(key >= tau) == 400 w.p. 1); when fewer
   than 400 pixels are masked the selection is garbage but v1 is False and
   the d1 term is discarded, matching the reference semantics.
"""

import numpy as np

TOP_K = 400
B, C, H, W = 16, 3, 768, 768
D, He, We = 64, 192, 192
N_CORES = 8

_compiled = {}


def _build():
    import jax
    import jax.numpy as jnp
    from jax.sharding import Mesh, PartitionSpec as P
    from jax.experimental.shard_map import shard_map

    devs = jax.devices()[:N_CORES]
    mesh = Mesh(np.asarray(devs), ("b",))

    def per_shard(outputs, embeddings, class_labels, outputs_aug, embeddings_aug):
        # shapes: [2, C, H, W], [2, D, He, We], [2, H, W] (local samples)
        nb = outputs.shape[0]

        def conf_seg_masks(o):
            # o: [nb, C, H, W] -> masks at downsampled res, flattened [nb, P]
            od = o[:, :, ::4, ::4]                              # [nb, C, He, We]
            e = jnp.exp(od)
            s = e.sum(axis=1)
            lse = jnp.log(s)
            mmax = od.max(axis=1)
            conf = jnp.exp(mmax - lse)
            c0, c1, c2 = od[:, 0], od[:, 1], od[:, 2]
            seg1 = (c1 > c0) & (c1 >= c2)
            seg2 = (c2 > c0) & (c2 > c1)
            m1 = seg1 & (conf > 0.8)
            m2 = seg2 & (conf > 0.6)
            return m1.reshape(nb, -1), m2.reshape(nb, -1)

        # ---- CE over full res (sample mean each) ----
        e_full = jnp.exp(outputs)
        lse_full = jnp.log(e_full.sum(axis=1))                  # [nb, H, W]
        lbl = class_labels
        x_lbl = jnp.where(lbl == 0, outputs[:, 0],
                          jnp.where(lbl == 1, outputs[:, 1], outputs[:, 2]))
        nll_sum = (lse_full - x_lbl).sum(axis=(1, 2))           # [nb]

        m1, m2 = conf_seg_masks(outputs)
        m1a, m2a = conf_seg_masks(outputs_aug)

        def emb_stats(emb):
            ef = emb.reshape(emb.shape[0], D, -1)               # [nb, D, P]
            norm = jnp.sqrt((ef * ef).sum(axis=1))              # [nb, P]
            nrm = ef / norm[:, None]
            key = ef.mean(axis=1)                               # [nb, P]
            return nrm, key

        nrm, key = emb_stats(embeddings)
        nrma, keya = emb_stats(embeddings_aug)

        def topk_threshold(keyd):
            # keyd: [nb, P] descending-order keys (-colmean; invalid = -1e20)
            kb = jnp.where(jnp.abs(keyd) < 1.0, keyd, jnp.sign(keyd))
            lo = jnp.full((keyd.shape[0],), -1.0, jnp.float32)
            hi = jnp.full((keyd.shape[0],), 1.0, jnp.float32)
            for _ in range(40):
                mid = 0.5 * (lo + hi)
                cnt = (kb >= mid[:, None]).sum(axis=1)
                ge = cnt >= TOP_K
                lo = jnp.where(ge, mid, lo)
                hi = jnp.where(ge, hi, mid)
            return lo

        def sel_sum(nrm_, key_, m1_):
            keyd = jnp.where(m1_, -key_, -1e20)
            tau = topk_threshold(keyd)
            sel = keyd >= tau[:, None]
            return (nrm_ * sel[:, None]).sum(axis=2) / TOP_K    # [nb, D]

        s1 = sel_sum(nrm, key, m1)
        s1a = sel_sum(nrma, keya, m1a)
        cnt1 = m1.sum(axis=1)
        cnt1a = m1a.sum(axis=1)

        def masked_mean(nrm_, m2_):
            cnt = m2_.sum(axis=1)
            mu = (nrm_ * m2_[:, None]).sum(axis=2) / jnp.maximum(cnt, 1)[:, None]
            return mu, cnt

        mu, cnt2 = masked_mean(nrm, m2)
        mua, cnt2a = masked_mean(nrma, m2a)

        d1 = 1.0 - (s1 * s1a).sum(axis=1)
        d2 = 1.0 - (mu * mua).sum(axis=1)
        stats = jnp.stack([
            nll_sum, d1, d2,
            cnt1.astype(jnp.float32), cnt1a.astype(jnp.float32),
            cnt2.astype(jnp.float32), cnt2a.astype(jnp.float32),
        ], axis=1)                                              # [nb, 7]
        return stats

    import jax
    fn = shard_map(
        per_shard, mesh=mesh,
        in_specs=(P("b"), P("b"), P("b"), P("b"), P("b")),
        out_specs=P("b"), check_rep=False,
    )
    return jax.jit(fn)


def kernel(outputs, embeddings, class_labels, outputs_aug, embeddings_aug,
           class_labels_aug=None, **_ignored):
    import jax.numpy as jnp
    if "fn" not in _compiled:
        _compiled["fn"] = _build()
    fn = _compiled["fn"]
    stats = np.asarray(fn(
        jnp.asarray(outputs), jnp.asarray(embeddings),
        jnp.asarray(class_labels.astype(np.int32)),
        jnp.asarray(outputs_aug), jnp.asarray(embeddings_aug),
    ))  # [B, 7]

    # ---- host-side gather/all-reduce of the per-sample scalars ----
    nll_sum, d1, d2 = stats[:, 0], stats[:, 1], stats[:, 2]
    cnt1, cnt1a, cnt2, cnt2a = stats[:, 3], stats[:, 4], stats[:, 5], stats[:, 6]
    loss_ce = np.float32((nll_sum / (H * W)).mean())
    v1 = (cnt1 > TOP_K) & (cnt1a > TOP_K)
    v2 = (cnt2 > 0) & (cnt2a > 0)
    cnt = v1.sum() + v2.sum()
    loss_matching = np.float32(
        (np.where(v1, d1, 0.0).sum() + np.where(v2, d2, 0.0).sum()) / max(cnt, 1))
    loss = np.float32(loss_ce + 2.0 * loss_matching)
    return np.asarray([loss, loss_ce, loss_matching], dtype=np.float32)
